# revision 11
# baseline (speedup 1.0000x reference)
"""Trainium2 Bass kernel for AnatomicalMaskedLinear (block-masked dense layer).

Reference op:
    mask  = kron(adjacency, ones(256, 128))            # (21*256, 21*128)
    y     = x.reshape(B, 21*128) @ (weight*mask).T + bias
    out   = y.reshape(B, 21, 256)

Strategy:
  * The mask zeroes whole (256 out x 128 in) blocks; blocks with A[i,j]==0
    contribute nothing, so only nonzero blocks are shipped/matmul'd.
  * 8 cores = 4 batch quarters x 2 node-row halves. Each node's 256 output
    rows share one adjacency row, so the two 128-row halves of every node
    block have identical sparsity structure -> all 8 cores run the SAME
    instruction schedule (one SPMD graph), only the data differs.
  * Per core: xT (2688 x 1024 f32), packed nonzero weight blocks
    ([128k x 128o] each, pre-transposed), bias slice. Device casts to fp16,
    accumulates in fp32 PSUM: out[o,b] += sum_k wT[k,o] * xT[k,b], adds bias,
    writes yT (2688 x 1024 f32). Host reassembles the full (4096, 21, 256).
  * Nodes are processed in a greedy order that minimizes new x-blocks early
    (fast time-to-first-matmul). Weight chunks stream per node (each block
    feeds exactly two matmuls, one per 512-batch half, interleaved so the
    stationary operand is reused). Input loads ride the Sync HWDGE queue;
    output stores + bias ride the Scalar HWDGE queue.
"""

import os
import numpy as np

NUM_NODES = 21
IN_F = 128
OUT_F = 256
BATCH = 4096
N_CORES = 8
P_BATCH = 4                      # batch ways
B_C = BATCH // P_BATCH           # 1024 batch rows per core
B_TILE = 512                     # matmul moving free dim
N_BT = B_C // B_TILE             # 2 batch tiles per core
K_TOTAL = NUM_NODES * IN_F       # 2688
O_C = NUM_NODES * 128            # 2688 out rows per core (half of each node)

_CACHE = {}                      # schedule key -> (nc, sched)


def _node_order(active):
    """Greedy: minimize newly-required x blocks at each step."""
    loaded = set()
    remaining = set(range(NUM_NODES))
    order = []
    while remaining:
        nxt = min(remaining,
                  key=lambda i: (len(set(active[i]) - loaded), len(active[i]), i))
        order.append(nxt)
        loaded |= set(active[nxt])
        remaining.remove(nxt)
    return order


def _build_schedule(adjacency):
    """[(i, [j...], zero_pad)] in greedy node order; >=1 slot per node."""
    A = np.asarray(adjacency) != 0
    active = {i: [int(j) for j in np.where(A[i])[0]] for i in range(NUM_NODES)}
    sched = []
    for i in _node_order(active):
        js = active[i]
        if js:
            sched.append((i, tuple(js), False))
        else:
            sched.append((i, (0,), True))
    return tuple(sched)


def _build_graph(sched):
    import concourse.tile as tile
    from concourse import bacc, mybir

    S = sum(len(js) for _, js, _ in sched)
    max_nnz = max(len(js) for _, js, _ in sched)
    f32 = mybir.dt.float32
    f16 = mybir.dt.float16

    nc = bacc.Bacc("TRN2", target_bir_lowering=False, debug=False,
                   num_devices=N_CORES)

    xt_d = nc.declare_dram_parameter("xt", [K_TOTAL, B_C], f16, isOutput=False)
    wp_d = nc.declare_dram_parameter("wp", [128, S * 128], f16, isOutput=False)
    bias_d = nc.declare_dram_parameter("biasr", [128, NUM_NODES], f32,
                                       isOutput=False)
    out_d = nc.declare_dram_parameter("out", [O_C, B_C], f32, isOutput=True)

    with tile.TileContext(nc) as tc:
        with (
            tc.tile_pool(name="const", bufs=1) as constp,
            tc.tile_pool(name="wbfp", bufs=3) as wbfp,
            tc.tile_pool(name="persist", bufs=1) as persist,
            tc.tile_pool(name="psum", bufs=8, space="PSUM") as psump,
            tc.tile_pool(name="outp", bufs=6) as outp,
        ):
            bias_sb = constp.tile([128, NUM_NODES], f32)
            nc.scalar.dma_start(out=bias_sb[:], in_=bias_d[:])

            xt_bf = persist.tile([128, NUM_NODES * B_C], f16)

            # per-node new-x lists, then emit with prefetch distance 1
            new_js = []
            seen = set()
            for i, js, _zero in sched:
                cur = [j for j in js if j not in seen]
                seen |= set(cur)
                new_js.append(cur)

            def load_x(node_idx):
                if node_idx == 0:
                    # halves, all bt0 first: the first accumulation group
                    # unblocks after ~0.3MB instead of ~1.7MB
                    for bt in range(N_BT):
                        for n_, j in enumerate(new_js[0]):
                            eng = (nc.sync if ((n_ + bt) % 2 == 0)
                                   else nc.scalar)
                            lo = bt * B_TILE
                            eng.dma_start(
                                out=xt_bf[:, j * B_C + lo:
                                          j * B_C + lo + B_TILE],
                                in_=xt_d[j * 128:(j + 1) * 128,
                                         lo:lo + B_TILE])
                    loaded_x.update(new_js[0])
                    return
                for j in new_js[node_idx]:
                    eng = nc.sync if (len(loaded_x) % 2 == 0) else nc.scalar
                    loaded_x.add(j)
                    eng.dma_start(out=xt_bf[:, j * B_C:(j + 1) * B_C],
                                  in_=xt_d[j * 128:(j + 1) * 128, :])

            loaded_x = set()
            s0 = 0
            for k, (i, js, _zero) in enumerate(sched):
                nj = len(js)
                wbf = wbfp.tile([128, max_nnz * 128], f16, tag="wbf")
                nc.sync.dma_start(out=wbf[:, :nj * 128],
                                  in_=wp_d[:, s0 * 128:(s0 + nj) * 128])
                if k == 0:
                    load_x(0)
                if k + 1 < len(sched):
                    load_x(k + 1)

                for bt in range(N_BT):
                    ps = psump.tile([128, B_TILE], f32, tag="acc",
                                    name=f"acc_{i}_{bt}")
                    for idx, j in enumerate(js):
                        nc.tensor.matmul(
                            ps[:],
                            wbf[:, idx * 128:(idx + 1) * 128],
                            xt_bf[:, j * B_C + bt * B_TILE:
                                  j * B_C + bt * B_TILE + B_TILE],
                            start=(idx == 0),
                            stop=(idx == nj - 1),
                        )
                    ot = outp.tile([128, B_TILE], f32, tag="ot")
                    nc.vector.tensor_scalar_add(ot[:], ps[:],
                                                bias_sb[:, i:i + 1])
                    nc.scalar.dma_start(
                        out=out_d[i * 128:(i + 1) * 128,
                                  bt * B_TILE:(bt + 1) * B_TILE],
                        in_=ot[:],
                    )
                s0 += nj

    nc.compile()
    return nc


def _get_graph(adjacency):
    sched = _build_schedule(adjacency)
    if sched not in _CACHE:
        _CACHE[sched] = (_build_graph(sched), sched)
    return _CACHE[sched]


def _pack_inputs(x, weight, bias, sched):
    """Build the 8 per-core input maps (host-side slicing/layout only)."""
    x = np.asarray(x, dtype=np.float32).reshape(BATCH, K_TOTAL).astype(np.float16)
    weight = np.asarray(weight, dtype=np.float32).astype(np.float16)
    bias = np.asarray(bias, dtype=np.float32)

    flat = []  # (i, j, zero) in slot order
    for i, js, zero in sched:
        for j in js:
            flat.append((i, j, zero))
    S = len(flat)

    w5 = weight.reshape(NUM_NODES, 2, 128, NUM_NODES, IN_F)  # i, h, o, j, k
    w5t = w5.transpose(1, 4, 0, 3, 2)                        # h, k, i, j, o

    si = np.array([f[0] for f in flat])
    sj = np.array([f[1] for f in flat])
    szero = np.array([f[2] for f in flat])

    wp_h = []
    for h in range(2):
        wp = np.ascontiguousarray(w5t[h][:, si, sj, :])      # [128, S, 128]
        if szero.any():
            wp[:, szero, :] = 0.0
        wp_h.append(wp.reshape(128, S * 128))

    bias3 = bias.reshape(NUM_NODES, 2, 128)
    bias_h = [np.ascontiguousarray(bias3[:, h, :].T) for h in range(2)]

    in_maps = []
    for c in range(N_CORES):
        bq, h = divmod(c, 2)
        xt = np.ascontiguousarray(x[bq * B_C:(bq + 1) * B_C].T)  # [2688, 1024]
        in_maps.append({
            "xt": xt,
            "wp": wp_h[h],
            "biasr": bias_h[h],
        })
    return in_maps


def _gather_output(results):
    y = np.empty((P_BATCH, B_C, NUM_NODES, 2, 128), dtype=np.float32)
    for c in range(N_CORES):
        bq, h = divmod(c, 2)
        oc = results[c]["out"].reshape(NUM_NODES, 128, B_C)
        y[bq, :, :, h, :] = oc.transpose(2, 0, 1)
    return y.reshape(BATCH, NUM_NODES, OUT_F)


def _ensure_axon_profile_hook():
    """Provide antenv.axon_hooks if the image lacks it (no-op otherwise).

    concourse.bass_utils imports antenv.axon_hooks on the trace path; some
    images miss the module, which would turn BASS_TRACE=1 into an
    ImportError. Registers the standard ctypes NTFF hook when possible.
    """
    try:
        import antenv.axon_hooks  # noqa: F401
        return
    except ImportError:
        pass
    try:
        import antenv
    except ImportError:
        return
    import contextlib
    import ctypes
    import sys
    import types

    hook = None
    try:
        lib = ctypes.CDLL("/opt/axon/libaxon_pjrt.so")
        if hasattr(lib, "axon_start_nrt_profile"):
            lib.axon_start_nrt_profile.argtypes = [
                ctypes.POINTER(ctypes.c_int64), ctypes.c_size_t]
            lib.axon_start_nrt_profile.restype = ctypes.c_int64
            lib.axon_stop_nrt_profile.argtypes = [ctypes.c_char_p]
            lib.axon_stop_nrt_profile.restype = ctypes.c_int64

            @contextlib.contextmanager
            def hook(output_dir, device_ids):
                import jax
                jax.devices()
                if device_ids:
                    ids = (ctypes.c_int64 * len(device_ids))(*device_ids)
                    rc = lib.axon_start_nrt_profile(ids, len(device_ids))
                else:
                    rc = lib.axon_start_nrt_profile(None, 0)
                if rc != 0:
                    raise RuntimeError(f"axon_start_nrt_profile rc={rc}")
                try:
                    yield
                finally:
                    lib.axon_stop_nrt_profile(str(output_dir).encode())
    except OSError:
        hook = None

    mod = types.ModuleType("antenv.axon_hooks")
    mod._hook = hook
    mod.get_axon_ntff_profile_hook = lambda: mod._hook

    def _set(h):
        mod._hook = h

    mod.set_axon_ntff_profile_hook = _set
    sys.modules["antenv.axon_hooks"] = mod
    antenv.axon_hooks = mod


def kernel(x, weight, bias, adjacency):
    from concourse.bass_utils import run_bass_kernel_spmd

    _ensure_axon_profile_hook()
    nc, sched = _get_graph(adjacency)
    in_maps = _pack_inputs(x, weight, bias, sched)

    kwargs = {}
    if os.environ.get("KERNEL_TRACE"):
        kwargs["trace"] = True
        tcores = os.environ.get("KERNEL_TRACE_CORES")
        if tcores:
            kwargs["trace_cores"] = [int(t) for t in tcores.split(",")]

    res = run_bass_kernel_spmd(nc, in_maps, core_ids=list(range(N_CORES)),
                               **kwargs)
    kernel.last_result = res
    return _gather_output(res.results)


kernel.last_result = None


# revision 12
# speedup vs baseline: 1.2142x; 1.2142x over previous
"""Trainium2 Bass kernel for AnatomicalMaskedLinear (block-masked dense layer).

Reference op:
    mask  = kron(adjacency, ones(256, 128))            # (21*256, 21*128)
    y     = x.reshape(B, 21*128) @ (weight*mask).T + bias
    out   = y.reshape(B, 21, 256)

Strategy:
  * The mask zeroes whole (256 out x 128 in) blocks; blocks with A[i,j]==0
    contribute nothing, so only nonzero blocks are shipped/matmul'd.
  * 8 cores = 4 batch quarters x 2 node-row halves. Each node's 256 output
    rows share one adjacency row, so the two 128-row halves of every node
    block have identical sparsity structure -> all 8 cores run the SAME
    instruction schedule (one SPMD graph), only the data differs.
  * Per core: xT (2688 x 1024 f32), packed nonzero weight blocks
    ([128k x 128o] each, pre-transposed), bias slice. Device casts to fp16,
    accumulates in fp32 PSUM: out[o,b] += sum_k wT[k,o] * xT[k,b], adds bias,
    writes yT (2688 x 1024 f32). Host reassembles the full (4096, 21, 256).
  * Nodes are processed in a greedy order that minimizes new x-blocks early
    (fast time-to-first-matmul). Weight chunks stream per node (each block
    feeds exactly two matmuls, one per 512-batch half, interleaved so the
    stationary operand is reused). Input loads ride the Sync HWDGE queue;
    output stores + bias ride the Scalar HWDGE queue.
"""

import os
import numpy as np

NUM_NODES = 21
IN_F = 128
OUT_F = 256
BATCH = 4096
N_CORES = 8
P_BATCH = 4                      # batch ways
B_C = BATCH // P_BATCH           # 1024 batch rows per core
B_TILE = 512                     # matmul moving free dim
N_BT = B_C // B_TILE             # 2 batch tiles per core
K_TOTAL = NUM_NODES * IN_F       # 2688
O_C = NUM_NODES * 128            # 2688 out rows per core (half of each node)

_CACHE = {}                      # schedule key -> (nc, sched)


def _node_order(active):
    """Greedy: minimize newly-required x blocks at each step."""
    loaded = set()
    remaining = set(range(NUM_NODES))
    order = []
    while remaining:
        nxt = min(remaining,
                  key=lambda i: (len(set(active[i]) - loaded), len(active[i]), i))
        order.append(nxt)
        loaded |= set(active[nxt])
        remaining.remove(nxt)
    return order


def _build_schedule(adjacency):
    """[(i, [j...], zero_pad)] in greedy node order; >=1 slot per node."""
    A = np.asarray(adjacency) != 0
    active = {i: [int(j) for j in np.where(A[i])[0]] for i in range(NUM_NODES)}
    sched = []
    for i in _node_order(active):
        js = active[i]
        if js:
            sched.append((i, tuple(js), False))
        else:
            sched.append((i, (0,), True))
    return tuple(sched)


def _build_graph(sched):
    import concourse.tile as tile
    from concourse import bacc, mybir

    S = sum(len(js) for _, js, _ in sched)
    max_nnz = max(len(js) for _, js, _ in sched)
    f32 = mybir.dt.float32
    f16 = mybir.dt.float16

    nc = bacc.Bacc("TRN2", target_bir_lowering=False, debug=False,
                   num_devices=N_CORES)

    xt_d = nc.declare_dram_parameter("xt", [K_TOTAL, B_C], f16, isOutput=False)
    wp_d = nc.declare_dram_parameter("wp", [128, S * 128], f16, isOutput=False)
    bias_d = nc.declare_dram_parameter("biasr", [128, NUM_NODES], f32,
                                       isOutput=False)
    out_d = nc.declare_dram_parameter("out", [O_C, B_C], f32, isOutput=True)

    with tile.TileContext(nc) as tc:
        with (
            tc.tile_pool(name="const", bufs=1) as constp,
            tc.tile_pool(name="wbfp", bufs=3) as wbfp,
            tc.tile_pool(name="persist", bufs=1) as persist,
            tc.tile_pool(name="psum", bufs=8, space="PSUM") as psump,
            tc.tile_pool(name="outp", bufs=6) as outp,
        ):
            bias_sb = constp.tile([128, NUM_NODES], f32)
            nc.scalar.dma_start(out=bias_sb[:], in_=bias_d[:])

            xt_bf = persist.tile([128, NUM_NODES * B_C], f16)

            # per-node new-x lists, then emit with prefetch distance 1
            new_js = []
            seen = set()
            for i, js, _zero in sched:
                cur = [j for j in js if j not in seen]
                seen |= set(cur)
                new_js.append(cur)

            def load_x(node_idx):
                for j in new_js[node_idx]:
                    eng = nc.sync if (len(loaded_x) % 2 == 0) else nc.scalar
                    loaded_x.add(j)
                    eng.dma_start(out=xt_bf[:, j * B_C:(j + 1) * B_C],
                                  in_=xt_d[j * 128:(j + 1) * 128, :])

            loaded_x = set()
            load_x(0)
            s0 = 0
            for k, (i, js, _zero) in enumerate(sched):
                nj = len(js)
                wbf = wbfp.tile([128, max_nnz * 128], f16, tag="wbf")
                nc.sync.dma_start(out=wbf[:, :nj * 128],
                                  in_=wp_d[:, s0 * 128:(s0 + nj) * 128])
                if k + 1 < len(sched):
                    load_x(k + 1)

                for bt in range(N_BT):
                    ps = psump.tile([128, B_TILE], f32, tag="acc",
                                    name=f"acc_{i}_{bt}")
                    for idx, j in enumerate(js):
                        nc.tensor.matmul(
                            ps[:],
                            wbf[:, idx * 128:(idx + 1) * 128],
                            xt_bf[:, j * B_C + bt * B_TILE:
                                  j * B_C + bt * B_TILE + B_TILE],
                            start=(idx == 0),
                            stop=(idx == nj - 1),
                        )
                    ot = outp.tile([128, B_TILE], f32, tag="ot")
                    nc.vector.tensor_scalar_add(ot[:], ps[:],
                                                bias_sb[:, i:i + 1])
                    nc.scalar.dma_start(
                        out=out_d[i * 128:(i + 1) * 128,
                                  bt * B_TILE:(bt + 1) * B_TILE],
                        in_=ot[:],
                    )
                s0 += nj

    nc.compile()
    return nc


def _get_graph(adjacency):
    sched = _build_schedule(adjacency)
    if sched not in _CACHE:
        _CACHE[sched] = (_build_graph(sched), sched)
    return _CACHE[sched]


def _pack_inputs(x, weight, bias, sched):
    """Build the 8 per-core input maps (host-side slicing/layout only)."""
    x = np.asarray(x, dtype=np.float32).reshape(BATCH, K_TOTAL).astype(np.float16)
    weight = np.asarray(weight, dtype=np.float32).astype(np.float16)
    bias = np.asarray(bias, dtype=np.float32)

    flat = []  # (i, j, zero) in slot order
    for i, js, zero in sched:
        for j in js:
            flat.append((i, j, zero))
    S = len(flat)

    w5 = weight.reshape(NUM_NODES, 2, 128, NUM_NODES, IN_F)  # i, h, o, j, k
    w5t = w5.transpose(1, 4, 0, 3, 2)                        # h, k, i, j, o

    si = np.array([f[0] for f in flat])
    sj = np.array([f[1] for f in flat])
    szero = np.array([f[2] for f in flat])

    wp_h = []
    for h in range(2):
        wp = np.ascontiguousarray(w5t[h][:, si, sj, :])      # [128, S, 128]
        if szero.any():
            wp[:, szero, :] = 0.0
        wp_h.append(wp.reshape(128, S * 128))

    bias3 = bias.reshape(NUM_NODES, 2, 128)
    bias_h = [np.ascontiguousarray(bias3[:, h, :].T) for h in range(2)]

    in_maps = []
    for c in range(N_CORES):
        bq, h = divmod(c, 2)
        xt = np.ascontiguousarray(x[bq * B_C:(bq + 1) * B_C].T)  # [2688, 1024]
        in_maps.append({
            "xt": xt,
            "wp": wp_h[h],
            "biasr": bias_h[h],
        })
    return in_maps


def _gather_output(results):
    y = np.empty((P_BATCH, B_C, NUM_NODES, 2, 128), dtype=np.float32)
    for c in range(N_CORES):
        bq, h = divmod(c, 2)
        oc = results[c]["out"].reshape(NUM_NODES, 128, B_C)
        y[bq, :, :, h, :] = oc.transpose(2, 0, 1)
    return y.reshape(BATCH, NUM_NODES, OUT_F)


def _ensure_axon_profile_hook():
    """Provide antenv.axon_hooks if the image lacks it (no-op otherwise).

    concourse.bass_utils imports antenv.axon_hooks on the trace path; some
    images miss the module, which would turn BASS_TRACE=1 into an
    ImportError. Registers the standard ctypes NTFF hook when possible.
    """
    try:
        import antenv.axon_hooks  # noqa: F401
        return
    except ImportError:
        pass
    try:
        import antenv
    except ImportError:
        return
    import contextlib
    import ctypes
    import sys
    import types

    hook = None
    try:
        lib = ctypes.CDLL("/opt/axon/libaxon_pjrt.so")
        if hasattr(lib, "axon_start_nrt_profile"):
            lib.axon_start_nrt_profile.argtypes = [
                ctypes.POINTER(ctypes.c_int64), ctypes.c_size_t]
            lib.axon_start_nrt_profile.restype = ctypes.c_int64
            lib.axon_stop_nrt_profile.argtypes = [ctypes.c_char_p]
            lib.axon_stop_nrt_profile.restype = ctypes.c_int64

            @contextlib.contextmanager
            def hook(output_dir, device_ids):
                import jax
                jax.devices()
                if device_ids:
                    ids = (ctypes.c_int64 * len(device_ids))(*device_ids)
                    rc = lib.axon_start_nrt_profile(ids, len(device_ids))
                else:
                    rc = lib.axon_start_nrt_profile(None, 0)
                if rc != 0:
                    raise RuntimeError(f"axon_start_nrt_profile rc={rc}")
                try:
                    yield
                finally:
                    lib.axon_stop_nrt_profile(str(output_dir).encode())
    except OSError:
        hook = None

    mod = types.ModuleType("antenv.axon_hooks")
    mod._hook = hook
    mod.get_axon_ntff_profile_hook = lambda: mod._hook

    def _set(h):
        mod._hook = h

    mod.set_axon_ntff_profile_hook = _set
    sys.modules["antenv.axon_hooks"] = mod
    antenv.axon_hooks = mod


def kernel(x, weight, bias, adjacency):
    from concourse.bass_utils import run_bass_kernel_spmd

    _ensure_axon_profile_hook()
    nc, sched = _get_graph(adjacency)
    in_maps = _pack_inputs(x, weight, bias, sched)

    kwargs = {}
    if os.environ.get("KERNEL_TRACE"):
        kwargs["trace"] = True
        tcores = os.environ.get("KERNEL_TRACE_CORES")
        if tcores:
            kwargs["trace_cores"] = [int(t) for t in tcores.split(",")]

    res = run_bass_kernel_spmd(nc, in_maps, core_ids=list(range(N_CORES)),
                               **kwargs)
    kernel.last_result = res
    return _gather_output(res.results)


kernel.last_result = None
